# revision 2
# baseline (speedup 1.0000x reference)
"""Trainium2 Bass kernel for nn_Contour_to_mask (winding-number soft
rasterization of a 128-point contour into a (1, 2, 256, 256) f32 mask).

Math: for pixel m = (mx, my) and contour edge (c_n, c_{n+1}):
  cross_n(m), dot_n(m) are bilinear in mesh features [1, mx, my, mx^2+my^2],
  so the TensorEngine computes them as K=4 matmuls against per-edge
  coefficient matrices (host-derived from the contour in float64).
  angle = arccos(clip(cos, -1+eps, 1-eps)) == pi/2 - arctan(clip(r, +-R1))
  with r = dot/|cross| (the cotangent of the angle), R1 = cot(arccos(1-eps)).
  contribution = tanh(1e5*cross) * angle; winding = |sum_n contrib|/2pi, clipped.

Engine split per 1024-pixel superblock (partitions = 128 contour points):
  PE:   cross/dot matmuls (K=4) + reduction-over-points matmuls using a
        sliding-window one-hot lhsT accumulating pi/2*sum(s) - sum(s*phi)
        for all 32 pixel-blocks into one [32, 512] PSUM tile.
  ACT:  s = tanh(1e5*cross)  and  phi = arctan(rc)   (same ACT table set)
  DVE:  v = reciprocal_approx_fast(cross)  (signed, ~3e-6 rel err) and a
        custom fused op rc = clip(dot*|v|, +-R1) with a select(m==m) NaN
        guard (cross==+-0 gives v=NaN; guard maps it to R1, and s=tanh(0)=0
        kills the contribution, matching the reference).
  GPSIMD: t2 = s * phi (inputs always finite; Q7 traps on NaN/Inf).

Sharding: 8 cores, each takes 1/8 of the (batch, pixel) grid: core c handles
batch c//4, image rows [(c%4)*64, +64) = 16384 pixels vs all 128 edges.
"""
import sys

sys.path.insert(0, "/opt/trn_rl_repo")

import numpy as np

SIZE = 256
K_TANH = 100000.0
EPS = 1e-5
B = 2
NPTS = 128
N_CORES = 8
PIX = SIZE * SIZE              # 65536
PIX_CORE = PIX * B // N_CORES  # 16384 (b, pixel) pairs per core
BLK = 512                      # pixels per reduction block (one PSUM bank)
NBLK = PIX_CORE // BLK         # 32
SBLK = 1024                    # pixels per elementwise superblock
NSBLK = PIX_CORE // SBLK       # 16

_compiled = None
_rmul_op = None


def _clip_bound():
    c = np.float64(np.float32(1.0 - EPS))
    return np.float32(c / np.sqrt(1.0 - c * c))


def _register_rmul_op():
    """Custom DVE op: out = clip(Src1*|Src0|, C1, C0), NaN -> C0."""
    global _rmul_op
    if _rmul_op is not None:
        return _rmul_op
    from concourse import dve_ops
    from concourse.dve_spec import (
        Spec, Src0, Src1, C0, C1, Zero, maxx, minn, eq, select, lower)
    from concourse.dve_uop import DveOpSpec

    name = "RMUL_CLIP_SAFE"
    if name in dve_ops._SUB_OPCODE_FOR_NAME:
        _rmul_op = next(op for op in dve_ops.OPS if op.name == name)
        return _rmul_op

    _av = maxx(Src0, Zero - Src0)
    _m = Src1 * _av
    _clipped = minn(maxx(_m, C1), C0)
    _body = select(eq(_m, _m), _clipped, C0)

    def _ref(in0, in1, s0, s1, imm2):
        m = in1 * np.abs(in0)
        out = np.minimum(np.maximum(m, s1), s0)
        return np.where(np.isnan(m), s0, out).astype(np.float32)

    spec = Spec(body=_body, reference=_ref)
    row = dve_ops._CUSTOM_DVE_ROW_BASE + len(dve_ops.OPS)
    sha = {ver: DveOpSpec(name=name, opcode=row, uops=lower(spec, ver=ver),
                          rd1_en=True).sha(ver)
           for ver in ("v3", "v4")}
    op = dve_ops.DveOp(name, spec, subdim=False, uops_sha=sha)
    dve_ops.OPS.append(op)
    dve_ops.CUSTOM_DVE_SPECS[name] = spec
    dve_ops._SUB_OPCODE_FOR_NAME[name] = row
    _rmul_op = op
    return op


def _build(repeat=1):
    import concourse.bacc as bacc
    import concourse.tile as tile
    import concourse.mybir as mybir

    AF = mybir.ActivationFunctionType
    ALU = mybir.AluOpType
    f32 = mybir.dt.float32
    u32 = mybir.dt.uint32
    rmul = _register_rmul_op()

    nc = bacc.Bacc("TRN2", target_bir_lowering=False, debug=False,
                   num_devices=N_CORES)

    feat_d = nc.dram_tensor("feat", [4, PIX_CORE], f32, kind="ExternalInput").ap()
    ccoef_d = nc.dram_tensor("ccoef", [4, NPTS], f32, kind="ExternalInput").ap()
    dcoef_d = nc.dram_tensor("dcoef", [4, NPTS], f32, kind="ExternalInput").ap()
    redp_d = nc.dram_tensor("redp", [NPTS, 63], f32, kind="ExternalInput").ap()
    redm_d = nc.dram_tensor("redm", [NPTS, 63], f32, kind="ExternalInput").ap()
    out_d = nc.dram_tensor("out", [NBLK, BLK], f32, kind="ExternalOutput").ap()

    R1 = float(_clip_bound())

    with tile.TileContext(nc) as tc:
        with tc.tile_pool(name="cst", bufs=1) as cst, \
             tc.tile_pool(name="work", bufs=3) as work, \
             tc.tile_pool(name="pcross", bufs=2, space="PSUM") as pcross, \
             tc.tile_pool(name="pdot", bufs=1, space="PSUM") as pdot, \
             tc.tile_pool(name="pacc", bufs=1, space="PSUM") as pacc:
            feat_t = cst.tile([4, PIX_CORE], f32, name="feat_t")
            ccoef_t = cst.tile([4, NPTS], f32, name="ccoef_t")
            dcoef_t = cst.tile([4, NPTS], f32, name="dcoef_t")
            redp_t = cst.tile([NPTS, 63], f32, name="redp_t")
            redm_t = cst.tile([NPTS, 63], f32, name="redm_t")
            nc.sync.dma_start(feat_t[:], feat_d[:])
            nc.sync.dma_start(ccoef_t[:], ccoef_d[:])
            nc.sync.dma_start(dcoef_t[:], dcoef_d[:])
            nc.sync.dma_start(redp_t[:], redp_d[:])
            nc.sync.dma_start(redm_t[:], redm_d[:])

            acc = pacc.tile([NBLK, BLK], f32, name="acc")

            for rep in range(repeat):
                for u in range(NSBLK):
                    cross = pcross.tile([NPTS, SBLK], f32, tag="cross",
                                        name=f"cross{rep}_{u}")
                    dot = pdot.tile([NPTS, SBLK], f32, tag="dot",
                                    name=f"dot{rep}_{u}")
                    for h in range(2):
                        fs = feat_t[:, u * SBLK + h * BLK:
                                    u * SBLK + (h + 1) * BLK]
                        hs = slice(h * BLK, (h + 1) * BLK)
                        nc.tensor.matmul(cross[:, hs], ccoef_t[:], fs,
                                         start=True, stop=True)
                        nc.tensor.matmul(dot[:, hs], dcoef_t[:], fs,
                                         start=True, stop=True)

                    s = work.tile([NPTS, SBLK], f32, tag="s", name=f"s{rep}_{u}")
                    nc.scalar.activation(s[:], cross[:], AF.Tanh, scale=K_TANH)

                    v = work.tile([NPTS, SBLK], f32, tag="v", name=f"v{rep}_{u}")
                    nc.vector.reciprocal_approx_fast(v[:], cross[:])

                    rc = work.tile([NPTS, SBLK], f32, tag="rc", name=f"rc{rep}_{u}")
                    nc.vector._custom_dve(rmul, out=rc[:], in0=v[:], in1=dot[:],
                                          s0=R1, s1=-R1)

                    phi = work.tile([NPTS, SBLK], f32, tag="phi",
                                    name=f"phi{rep}_{u}")
                    nc.scalar.activation(phi[:], rc[:], AF.Arctan)

                    t2 = work.tile([NPTS, SBLK], f32, tag="t2", name=f"t2{rep}_{u}")
                    nc.gpsimd.tensor_tensor(t2[:], s[:], phi[:], ALU.mult)

                    for h in range(2):
                        j = 2 * u + h
                        hs = slice(h * BLK, (h + 1) * BLK)
                        lp = redp_t[:, 31 - j:63 - j]
                        lm = redm_t[:, 31 - j:63 - j]
                        nc.tensor.matmul(acc[:], lp, s[:, hs],
                                         start=(j == 0), stop=False)
                        nc.tensor.matmul(acc[:], lm, t2[:, hs], start=False,
                                         stop=(j == NBLK - 1 and
                                               rep == repeat - 1))

            absd = work.tile([NBLK, BLK], f32, tag="absd", name="absd")
            nc.vector.tensor_scalar(
                absd[:].bitcast(u32), acc[:].bitcast(u32),
                0x7FFFFFFF, None, ALU.bitwise_and)
            w = work.tile([NBLK, BLK], f32, tag="w", name="w")
            nc.vector.tensor_scalar(
                w[:], absd[:], float(np.float32(1.0 / (2.0 * np.pi))), 1.0,
                ALU.mult, ALU.min)
            nc.sync.dma_start(out_d[:], w[:])

    nc.compile()
    return nc


def _host_inputs(contour: np.ndarray):
    """Per-core in_maps from the full (B, NPTS, 2) contour."""
    i_idx, j_idx = np.meshgrid(np.arange(SIZE), np.arange(SIZE), indexing="ij")
    mx = (i_idx / SIZE).reshape(-1).astype(np.float32)
    my = (j_idx / SIZE).reshape(-1).astype(np.float32)
    feat = np.stack([
        np.ones_like(mx), mx, my,
        (mx.astype(np.float64) ** 2 + my.astype(np.float64) ** 2).astype(np.float32),
    ])  # [4, PIX]

    ccoefs, dcoefs = [], []
    for b in range(B):
        cx = contour[b, :, 0].astype(np.float64)
        cy = contour[b, :, 1].astype(np.float64)
        cxn = np.roll(cx, -1)
        cyn = np.roll(cy, -1)
        ccoefs.append(np.stack([cy * cxn - cx * cyn, cyn - cy, cx - cxn,
                                np.zeros(NPTS)]).astype(np.float32))
        dcoefs.append(np.stack([cx * cxn + cy * cyn, -(cx + cxn), -(cy + cyn),
                                np.ones(NPTS)]).astype(np.float32))

    redp = np.zeros((NPTS, 63), dtype=np.float32)
    redp[:, 31] = np.float32(np.pi / 2)
    redm = np.zeros((NPTS, 63), dtype=np.float32)
    redm[:, 31] = np.float32(-1.0)

    in_maps = []
    per_batch = PIX // (N_CORES // B)  # 16384
    for c in range(N_CORES):
        b = c // (N_CORES // B)
        lo = (c % (N_CORES // B)) * per_batch
        in_maps.append({
            "feat": np.ascontiguousarray(feat[:, lo:lo + per_batch]),
            "ccoef": ccoefs[b],
            "dcoef": dcoefs[b],
            "redp": redp,
            "redm": redm,
        })
    return in_maps


def kernel(contour: np.ndarray) -> np.ndarray:
    global _compiled
    from concourse import bass_utils

    contour = np.asarray(contour, dtype=np.float32)
    if _compiled is None:
        _compiled = _build()
    in_maps = _host_inputs(contour)
    res = bass_utils.run_bass_kernel_spmd(
        _compiled, in_maps, core_ids=list(range(N_CORES))).results

    mask = np.zeros((1, B, SIZE, SIZE), dtype=np.float32)
    per_batch = PIX // (N_CORES // B)
    rows_per_core = per_batch // SIZE  # 64 image rows
    for c in range(N_CORES):
        b = c // (N_CORES // B)
        r0 = (c % (N_CORES // B)) * rows_per_core
        mask[0, b, r0:r0 + rows_per_core, :] = (
            res[c]["out"].reshape(rows_per_core, SIZE))
    return mask


# revision 7
# speedup vs baseline: 206.6178x; 206.6178x over previous
"""Trainium2 Bass kernel for nn_Contour_to_mask (winding-number soft
rasterization of a 128-point contour into a (1, 2, 256, 256) f32 mask).

Math: for pixel m = (mx, my) = (i/256, j/256) and edge (c_n, c_{n+1}):
  cross_n(m) = (cy*cxn - cx*cyn) + (cyn-cy)*mx + (cx-cxn)*my
  dot_n(m)   = (cx*cxn + cy*cyn) - (cx+cxn)*mx - (cy+cyn)*my + mx^2 + my^2
Both are SEPARABLE into per-edge row/column profiles:
  cross[n, i, j] = Pc[n, i] + Qc[n, j];   dot[n, i, j] = Rd[n, i] + Sd[n, j].
  angle = arccos(clip(cos, -1+eps, 1-eps)) == pi/2 - arctan(clip(r, +-R1))
  with r = dot/|cross|, R1 = cot(arccos(1-eps)).
  contribution = tanh(1e5*cross)*angle; winding = |sum_n contrib|/2pi, clip.

Engine split per 2048-pixel (8-image-row) superblock (partitions = 128 edges):
  ACT:  4 cross row-builds (Identity w/ per-partition bias), s = tanh(1e5*
        cross) -> bf16, phi = arctan(rc). Tanh+Arctan share one table set.
  GPSIMD: 4 cross row-builds (tensor_scalar add), t2 = s*phi.
  DVE:  v = reciprocal_approx_fast(cross); per-row custom fused op
        rc = clip((Sd + Rd_i)*|v|, +-R1) that BUILDS dot inline (Sd tensor +
        per-partition scalar Rd_i) and guards NaN via select(m==m)
        (cross==+-0 -> v=NaN -> rc:=R1; s=tanh(0)=0 kills it, matching ref).
  PE:   reduction over the 128 edges via sliding-window one-hot lhsT matmuls
        into two PSUM tiles: accS = sum(s) (bf16 rhs, full-rate) and
        accT = sum(t2) (fp32 rhs); finale w = min(|pi/2*accS - accT|/2pi, 1).

Sharding: 8 cores; core c handles batch c//4, image rows [(c%4)*64, +64).
"""
import sys

sys.path.insert(0, "/opt/trn_rl_repo")

import numpy as np

SIZE = 256
K_TANH = 100000.0
EPS = 1e-5
B = 2
NPTS = 128
N_CORES = 8
PIX = SIZE * SIZE              # 65536
PIX_CORE = PIX * B // N_CORES  # 16384 pixels per core
ROWS_CORE = PIX_CORE // SIZE   # 64 image rows per core
BLK = 512                      # pixels per reduction block (one PSUM bank)
NBLK = PIX_CORE // BLK         # 32
SBLK = 2048                    # pixels per elementwise superblock (8 rows)
NSBLK = PIX_CORE // SBLK       # 8

_compiled = {}
_ops = {}


def _clip_bound():
    c = np.float64(np.float32(1.0 - EPS))
    return np.float32(c / np.sqrt(1.0 - c * c))


def _register_ops():
    """Register the two custom DVE ops (idempotent)."""
    if _ops:
        return _ops
    from concourse import dve_ops
    from concourse.dve_spec import (
        Spec, Src0, Src1, C0, C1, C2, Zero, maxx, minn, eq, select, lower)
    from concourse.dve_uop import DveOpSpec

    def reg(name, spec):
        if name in dve_ops._SUB_OPCODE_FOR_NAME:
            return next(op for op in dve_ops.OPS if op.name == name)
        row = dve_ops._CUSTOM_DVE_ROW_BASE + len(dve_ops.OPS)
        sha = {ver: DveOpSpec(name=name, opcode=row,
                              uops=lower(spec, ver=ver), rd1_en=True).sha(ver)
               for ver in ("v3", "v4")}
        op = dve_ops.DveOp(name, spec, subdim=False, uops_sha=sha)
        dve_ops.OPS.append(op)
        dve_ops.CUSTOM_DVE_SPECS[name] = spec
        dve_ops._SUB_OPCODE_FOR_NAME[name] = row
        return op

    # DOT_RMUL_CLIP: out = clip((Src1 + C0)*|Src0|, -C1, C1); NaN -> C1.
    # Src0 = v (recip of cross), Src1 = Sd column profile, C0 = Rd_i scalar.
    _d = Src1 + C0
    _av = maxx(Src0, Zero - Src0)
    _m = _d * _av
    _cl = minn(maxx(_m, Zero - C1), C1)
    _body = select(eq(_m, _m), _cl, C1)

    def _ref_rmul(in0, in1, s0, s1, imm2):
        m = (in1 + s0) * np.abs(in0)
        out = np.minimum(np.maximum(m, -s1), s1)
        return np.where(np.isnan(m), s1, out).astype(np.float32)

    _ops["rmul"] = reg("DOT_RMUL_CLIP", Spec(body=_body, reference=_ref_rmul))

    # FINALE: out = min(|Src0*C0 - Src1| * C1, C2)
    _fd = Src0 * C0 - Src1
    _fa = maxx(_fd, Zero - _fd)
    _fbody = minn(_fa * C1, C2)

    def _ref_fin(in0, in1, s0, s1, imm2):
        return np.minimum(np.abs(in0 * s0 - in1) * s1, imm2).astype(np.float32)

    _ops["fin"] = reg("WINDING_FINALE", Spec(body=_fbody, reference=_ref_fin))
    return _ops


def _build(repeat=1):
    import concourse.bacc as bacc
    import concourse.tile as tile
    import concourse.mybir as mybir

    AF = mybir.ActivationFunctionType
    ALU = mybir.AluOpType
    f32 = mybir.dt.float32
    bf16 = mybir.dt.bfloat16
    ops = _register_ops()

    nc = bacc.Bacc("TRN2", target_bir_lowering=False, debug=False,
                   num_devices=N_CORES)

    pc_d = nc.dram_tensor("pc", [NPTS, ROWS_CORE], f32, kind="ExternalInput").ap()
    qc_d = nc.dram_tensor("qc", [NPTS, SIZE], f32, kind="ExternalInput").ap()
    rd_d = nc.dram_tensor("rd", [NPTS, ROWS_CORE], f32, kind="ExternalInput").ap()
    sd_d = nc.dram_tensor("sd", [NPTS, SIZE], f32, kind="ExternalInput").ap()
    redp_d = nc.dram_tensor("redp", [NPTS, 63], bf16, kind="ExternalInput").ap()
    redm_d = nc.dram_tensor("redm", [NPTS, 63], f32, kind="ExternalInput").ap()
    out_d = nc.dram_tensor("out", [NBLK, BLK], f32, kind="ExternalOutput").ap()

    R1 = float(_clip_bound())
    RPB = SBLK // SIZE  # rows per superblock = 8
    BPB = SBLK // BLK   # reduction blocks per superblock = 4

    with tile.TileContext(nc) as tc:
        with tc.tile_pool(name="cst", bufs=1) as cst, \
             tc.tile_pool(name="work", bufs=3) as work, \
             tc.tile_pool(name="pacc", bufs=1, space="PSUM") as pacc:
            pc_t = cst.tile([NPTS, ROWS_CORE], f32, name="pc_t")
            qc_t = cst.tile([NPTS, SIZE], f32, name="qc_t")
            rd_t = cst.tile([NPTS, ROWS_CORE], f32, name="rd_t")
            sd_t = cst.tile([NPTS, SIZE], f32, name="sd_t")
            redp_t = cst.tile([NPTS, 63], bf16, name="redp_t")
            redm_t = cst.tile([NPTS, 63], f32, name="redm_t")
            nc.sync.dma_start(pc_t[:], pc_d[:])
            nc.sync.dma_start(qc_t[:], qc_d[:])
            nc.sync.dma_start(rd_t[:], rd_d[:])
            nc.sync.dma_start(sd_t[:], sd_d[:])
            nc.sync.dma_start(redp_t[:], redp_d[:])
            nc.sync.dma_start(redm_t[:], redm_d[:])

            accS = pacc.tile([NBLK, BLK], f32, name="accS")
            accT = pacc.tile([NBLK, BLK], f32, name="accT")

            for rep in range(repeat):
                for u in range(NSBLK):
                    cross = work.tile([NPTS, SBLK], f32, tag="cross",
                                      name=f"cross{rep}_{u}")
                    for h in range(RPB):
                        i = u * RPB + h  # local image row
                        hs = slice(h * SIZE, (h + 1) * SIZE)
                        if h % 2 == 1:
                            nc.gpsimd.tensor_scalar(
                                cross[:, hs], qc_t[:], pc_t[:, i:i + 1], None,
                                ALU.add)
                        else:
                            nc.scalar.activation(
                                cross[:, hs], qc_t[:], AF.Identity,
                                bias=pc_t[:, i:i + 1])

                    s = work.tile([NPTS, SBLK], bf16, tag="s", name=f"s{rep}_{u}")
                    nc.scalar.activation(s[:], cross[:], AF.Tanh, scale=K_TANH)

                    v = work.tile([NPTS, SBLK], f32, tag="v", name=f"v{rep}_{u}")
                    nc.vector.reciprocal_approx_fast(v[:], cross[:])

                    rc = work.tile([NPTS, SBLK], f32, tag="rc", name=f"rc{rep}_{u}")
                    for h in range(RPB):
                        i = u * RPB + h
                        hs = slice(h * SIZE, (h + 1) * SIZE)
                        nc.vector._custom_dve(
                            ops["rmul"], out=rc[:, hs], in0=v[:, hs],
                            in1=sd_t[:], s0=rd_t[:, i:i + 1], s1=R1)

                    phi = work.tile([NPTS, SBLK], f32, tag="phi",
                                    name=f"phi{rep}_{u}")
                    nc.scalar.activation(phi[:], rc[:], AF.Arctan)

                    t2 = work.tile([NPTS, SBLK], f32, tag="t2", name=f"t2{rep}_{u}")
                    nc.gpsimd.tensor_tensor(t2[:], s[:], phi[:], ALU.mult)

                    for h in range(BPB):
                        j = BPB * u + h
                        hs = slice(h * BLK, (h + 1) * BLK)
                        lp = redp_t[:, 31 - j:63 - j]
                        lm = redm_t[:, 31 - j:63 - j]
                        nc.tensor.matmul(accS[:], lp, s[:, hs],
                                         start=(j == 0), stop=False)
                        nc.tensor.matmul(accT[:], lm, t2[:, hs],
                                         start=(j == 0),
                                         stop=(j == NBLK - 1 and
                                               rep == repeat - 1))

            tcopy = work.tile([NBLK, BLK], f32, tag="tcopy", name="tcopy")
            nc.vector.tensor_copy(tcopy[:], accT[:])
            w = work.tile([NBLK, BLK], f32, tag="w", name="w")
            nc.vector._custom_dve(
                ops["fin"], out=w[:], in0=accS[:], in1=tcopy[:],
                s0=float(np.float32(np.pi / 2)),
                s1=float(np.float32(1.0 / (2.0 * np.pi))), imm2=1.0)
            nc.sync.dma_start(out_d[:], w[:])

    nc.compile()
    return nc


def _host_inputs(contour: np.ndarray):
    """Per-core in_maps from the full (B, NPTS, 2) contour."""
    mx = (np.arange(SIZE) / SIZE).astype(np.float64)   # i profile
    my = (np.arange(SIZE) / SIZE).astype(np.float64)   # j profile

    prof = []
    for b in range(B):
        cx = contour[b, :, 0].astype(np.float64)
        cy = contour[b, :, 1].astype(np.float64)
        cxn = np.roll(cx, -1)
        cyn = np.roll(cy, -1)
        A = cy * cxn - cx * cyn
        Bc = cyn - cy
        Cc = cx - cxn
        Dd = cx * cxn + cy * cyn
        Ed = -(cx + cxn)
        Fd = -(cy + cyn)
        Pc = (A[:, None] + Bc[:, None] * mx[None, :]).astype(np.float32)
        Qc = (Cc[:, None] * my[None, :]).astype(np.float32)
        Rd = (Dd[:, None] + Ed[:, None] * mx[None, :] + mx[None, :] ** 2
              ).astype(np.float32)
        Sd = (Fd[:, None] * my[None, :] + my[None, :] ** 2).astype(np.float32)
        prof.append((Pc, Qc, Rd, Sd))

    import ml_dtypes
    redp = np.zeros((NPTS, 63), dtype=ml_dtypes.bfloat16)
    redp[:, 31] = 1.0
    redm = np.zeros((NPTS, 63), dtype=np.float32)
    redm[:, 31] = 1.0

    in_maps = []
    for c in range(N_CORES):
        b = c // (N_CORES // B)
        r0 = (c % (N_CORES // B)) * ROWS_CORE
        Pc, Qc, Rd, Sd = prof[b]
        in_maps.append({
            "pc": np.ascontiguousarray(Pc[:, r0:r0 + ROWS_CORE]),
            "qc": Qc,
            "rd": np.ascontiguousarray(Rd[:, r0:r0 + ROWS_CORE]),
            "sd": Sd,
            "redp": redp,
            "redm": redm,
        })
    return in_maps


def kernel(contour: np.ndarray) -> np.ndarray:
    from concourse import bass_utils

    contour = np.asarray(contour, dtype=np.float32)
    if "nc" not in _compiled:
        _compiled["nc"] = _build()
    in_maps = _host_inputs(contour)
    res = bass_utils.run_bass_kernel_spmd(
        _compiled["nc"], in_maps, core_ids=list(range(N_CORES))).results

    mask = np.zeros((1, B, SIZE, SIZE), dtype=np.float32)
    for c in range(N_CORES):
        b = c // (N_CORES // B)
        r0 = (c % (N_CORES // B)) * ROWS_CORE
        mask[0, b, r0:r0 + ROWS_CORE, :] = (
            res[c]["out"].reshape(ROWS_CORE, SIZE))
    return mask


# revision 11
# speedup vs baseline: 217.3794x; 1.0521x over previous
"""Trainium2 Bass kernel for nn_Contour_to_mask (winding-number soft
rasterization of a 128-point contour into a (1, 2, 256, 256) f32 mask).

Math: for pixel m = (mx, my) = (i/256, j/256) and edge (c_n, c_{n+1}):
  cross_n(m) = (cy*cxn - cx*cyn) + (cyn-cy)*mx + (cx-cxn)*my
  dot_n(m)   = (cx*cxn + cy*cyn) - (cx+cxn)*mx - (cy+cyn)*my + mx^2 + my^2
Both are SEPARABLE into per-edge row/column profiles:
  cross[n, i, j] = Pc[n, i] + Qc[n, j];   dot[n, i, j] = Rd[n, i] + Sd[n, j].
  angle = arccos(clip(cos, -1+eps, 1-eps)) == pi/2 - arctan(clip(r, +-R1))
  with r = dot/|cross|, R1 = cot(arccos(1-eps)).
  contribution = tanh(1e5*cross)*angle; winding = |sum_n contrib|/2pi, clip.

Engine split per 2048-pixel (8-image-row) superblock (partitions = 128 edges):
  ACT:  4 cross row-builds (Identity w/ per-partition bias), s = tanh(1e5*
        cross) -> bf16, phi = arctan(rc). Tanh+Arctan share one table set.
  GPSIMD: 4 cross row-builds (tensor_scalar add), t2 = s*phi.
  DVE:  v = reciprocal_approx_fast(cross); per-row custom fused op
        rc = clip((Sd + Rd_i)*|v|, +-R1) that BUILDS dot inline (Sd tensor +
        per-partition scalar Rd_i) and guards NaN via select(m==m)
        (cross==+-0 -> v=NaN -> rc:=R1; s=tanh(0)=0 kills it, matching ref).
  PE:   reduction over the 128 edges via sliding-window one-hot lhsT matmuls
        into two PSUM tiles: accS = sum(s) (bf16 rhs, full-rate) and
        accT = sum(t2) (fp32 rhs); finale w = min(|pi/2*accS - accT|/2pi, 1).

Sharding: 8 cores; core c handles batch c//4, image rows [(c%4)*64, +64).
"""
import sys

sys.path.insert(0, "/opt/trn_rl_repo")

import numpy as np

SIZE = 256
K_TANH = 100000.0
EPS = 1e-5
B = 2
NPTS = 128
N_CORES = 8
PIX = SIZE * SIZE              # 65536
PIX_CORE = PIX * B // N_CORES  # 16384 pixels per core
ROWS_CORE = PIX_CORE // SIZE   # 64 image rows per core
BLK = 512                      # pixels per reduction block (one PSUM bank)
NBLK = PIX_CORE // BLK         # 32
SBLK = 2048                    # pixels per elementwise superblock (8 rows)
NSBLK = PIX_CORE // SBLK       # 8

_compiled = {}
_ops = {}


def _clip_bound():
    c = np.float64(np.float32(1.0 - EPS))
    return np.float32(c / np.sqrt(1.0 - c * c))


def _register_ops():
    """Register the two custom DVE ops (idempotent)."""
    if _ops:
        return _ops
    from concourse import dve_ops
    from concourse.dve_spec import (
        Spec, Src0, Src1, C0, C1, C2, Zero, maxx, minn, eq, select, lower)
    from concourse.dve_uop import DveOpSpec

    def reg(name, spec):
        if name in dve_ops._SUB_OPCODE_FOR_NAME:
            return next(op for op in dve_ops.OPS if op.name == name)
        row = dve_ops._CUSTOM_DVE_ROW_BASE + len(dve_ops.OPS)
        sha = {ver: DveOpSpec(name=name, opcode=row,
                              uops=lower(spec, ver=ver), rd1_en=True).sha(ver)
               for ver in ("v3", "v4")}
        op = dve_ops.DveOp(name, spec, subdim=False, uops_sha=sha)
        dve_ops.OPS.append(op)
        dve_ops.CUSTOM_DVE_SPECS[name] = spec
        dve_ops._SUB_OPCODE_FOR_NAME[name] = row
        return op

    # DOT_RMUL_CLIP: out = clip((Src1 + C0)*|Src0|, -C1, C1); NaN -> C1.
    # Src0 = v (recip of cross), Src1 = Sd column profile, C0 = Rd_i scalar.
    _d = Src1 + C0
    _av = maxx(Src0, Zero - Src0)
    _m = _d * _av
    _cl = minn(maxx(_m, Zero - C1), C1)
    _body = select(eq(_m, _m), _cl, C1)

    def _ref_rmul(in0, in1, s0, s1, imm2):
        m = (in1 + s0) * np.abs(in0)
        out = np.minimum(np.maximum(m, -s1), s1)
        return np.where(np.isnan(m), s1, out).astype(np.float32)

    _ops["rmul"] = reg("DOT_RMUL_CLIP", Spec(body=_body, reference=_ref_rmul))

    # FINALE: out = min(|Src0*C0 - Src1| * C1, C2)
    _fd = Src0 * C0 - Src1
    _fa = maxx(_fd, Zero - _fd)
    _fbody = minn(_fa * C1, C2)

    def _ref_fin(in0, in1, s0, s1, imm2):
        return np.minimum(np.abs(in0 * s0 - in1) * s1, imm2).astype(np.float32)

    _ops["fin"] = reg("WINDING_FINALE", Spec(body=_fbody, reference=_ref_fin))
    return _ops


def _build(repeat=1):
    import concourse.bacc as bacc
    import concourse.tile as tile
    import concourse.mybir as mybir

    AF = mybir.ActivationFunctionType
    ALU = mybir.AluOpType
    f32 = mybir.dt.float32
    bf16 = mybir.dt.bfloat16
    ops = _register_ops()

    nc = bacc.Bacc("TRN2", target_bir_lowering=False, debug=False,
                   num_devices=N_CORES)

    pc_d = nc.dram_tensor("pc", [NPTS, ROWS_CORE], f32, kind="ExternalInput").ap()
    qc_d = nc.dram_tensor("qc", [NPTS, SIZE], f32, kind="ExternalInput").ap()
    rd_d = nc.dram_tensor("rd", [NPTS, ROWS_CORE], f32, kind="ExternalInput").ap()
    sd_d = nc.dram_tensor("sd", [NPTS, SIZE], f32, kind="ExternalInput").ap()
    redp_d = nc.dram_tensor("redp", [NPTS, 63], bf16, kind="ExternalInput").ap()
    redm_d = nc.dram_tensor("redm", [NPTS, 63], f32, kind="ExternalInput").ap()
    out_d = nc.dram_tensor("out", [NBLK, BLK], f32, kind="ExternalOutput").ap()

    R1 = float(_clip_bound())
    RPB = SBLK // SIZE  # rows per superblock = 8
    BPB = SBLK // BLK   # reduction blocks per superblock = 4

    with tile.TileContext(nc) as tc:
        with tc.tile_pool(name="cst", bufs=1) as cst, \
             tc.tile_pool(name="work", bufs=3) as work, \
             tc.tile_pool(name="pacc", bufs=1, space="PSUM") as pacc:
            pc_t = cst.tile([NPTS, ROWS_CORE], f32, name="pc_t")
            qc_t = cst.tile([NPTS, SIZE], f32, name="qc_t")
            rd_t = cst.tile([NPTS, ROWS_CORE], f32, name="rd_t")
            sd_t = cst.tile([NPTS, SIZE], f32, name="sd_t")
            redp_t = cst.tile([NPTS, 63], bf16, name="redp_t")
            redm_t = cst.tile([NPTS, 63], f32, name="redm_t")
            nc.sync.dma_start(pc_t[:], pc_d[:])
            nc.sync.dma_start(qc_t[:], qc_d[:])
            nc.sync.dma_start(rd_t[:], rd_d[:])
            nc.sync.dma_start(sd_t[:], sd_d[:])
            nc.sync.dma_start(redp_t[:], redp_d[:])
            nc.sync.dma_start(redm_t[:], redm_d[:])

            accS = pacc.tile([NBLK, BLK], f32, name="accS")
            accT = pacc.tile([NBLK, BLK], f32, name="accT")

            for rep in range(repeat):
                for u in range(NSBLK):
                    cross = work.tile([NPTS, SBLK], f32, tag="cross",
                                      name=f"cross{rep}_{u}")
                    for h in range(RPB):
                        i = u * RPB + h  # local image row
                        hs = slice(h * SIZE, (h + 1) * SIZE)
                        if h % 2 == 1:
                            nc.gpsimd.tensor_scalar(
                                cross[:, hs], qc_t[:], pc_t[:, i:i + 1], None,
                                ALU.add)
                        elif h >= 4:
                            nc.vector.tensor_scalar(
                                cross[:, hs], qc_t[:], pc_t[:, i:i + 1], None,
                                ALU.add)
                        else:
                            nc.scalar.activation(
                                cross[:, hs], qc_t[:], AF.Identity,
                                bias=pc_t[:, i:i + 1])

                    s = work.tile([NPTS, SBLK], bf16, tag="s", name=f"s{rep}_{u}")
                    for g in range(2):
                        gs = slice(g * (SBLK // 2), (g + 1) * (SBLK // 2))
                        nc.scalar.activation(s[:, gs], cross[:, gs], AF.Tanh,
                                             scale=K_TANH)

                    v = work.tile([NPTS, SBLK], f32, tag="v", name=f"v{rep}_{u}")
                    for g in range(2):
                        gs = slice(g * (SBLK // 2), (g + 1) * (SBLK // 2))
                        nc.vector.reciprocal_approx_fast(v[:, gs], cross[:, gs])

                    rc = work.tile([NPTS, SBLK], f32, tag="rc", name=f"rc{rep}_{u}")
                    for h in range(RPB):
                        i = u * RPB + h
                        hs = slice(h * SIZE, (h + 1) * SIZE)
                        nc.vector._custom_dve(
                            ops["rmul"], out=rc[:, hs], in0=v[:, hs],
                            in1=sd_t[:], s0=rd_t[:, i:i + 1], s1=R1)

                    phi = work.tile([NPTS, SBLK], f32, tag="phi",
                                    name=f"phi{rep}_{u}")
                    for g in range(2):
                        gs = slice(g * (SBLK // 2), (g + 1) * (SBLK // 2))
                        nc.scalar.activation(phi[:, gs], rc[:, gs], AF.Arctan)

                    t2 = work.tile([NPTS, SBLK], f32, tag="t2", name=f"t2{rep}_{u}")
                    for g in range(4):
                        gs = slice(g * (SBLK // 4), (g + 1) * (SBLK // 4))
                        nc.gpsimd.tensor_tensor(t2[:, gs], s[:, gs], phi[:, gs],
                                                ALU.mult)

                    for h in range(BPB):
                        j = BPB * u + h
                        hs = slice(h * BLK, (h + 1) * BLK)
                        lp = redp_t[:, 31 - j:63 - j]
                        lm = redm_t[:, 31 - j:63 - j]
                        nc.tensor.matmul(accS[:], lp, s[:, hs],
                                         start=(j == 0), stop=False)
                        nc.tensor.matmul(accT[:], lm, t2[:, hs],
                                         start=(j == 0),
                                         stop=(j == NBLK - 1 and
                                               rep == repeat - 1))

            tcopy = work.tile([NBLK, BLK], f32, tag="tcopy", name="tcopy")
            nc.vector.tensor_copy(tcopy[:], accT[:])
            w = work.tile([NBLK, BLK], f32, tag="w", name="w")
            nc.vector._custom_dve(
                ops["fin"], out=w[:], in0=accS[:], in1=tcopy[:],
                s0=float(np.float32(np.pi / 2)),
                s1=float(np.float32(1.0 / (2.0 * np.pi))), imm2=1.0)
            nc.sync.dma_start(out_d[:], w[:])

    nc.compile()
    return nc


def _host_inputs(contour: np.ndarray):
    """Per-core in_maps from the full (B, NPTS, 2) contour."""
    mx = (np.arange(SIZE) / SIZE).astype(np.float64)   # i profile
    my = (np.arange(SIZE) / SIZE).astype(np.float64)   # j profile

    prof = []
    for b in range(B):
        cx = contour[b, :, 0].astype(np.float64)
        cy = contour[b, :, 1].astype(np.float64)
        cxn = np.roll(cx, -1)
        cyn = np.roll(cy, -1)
        A = cy * cxn - cx * cyn
        Bc = cyn - cy
        Cc = cx - cxn
        Dd = cx * cxn + cy * cyn
        Ed = -(cx + cxn)
        Fd = -(cy + cyn)
        Pc = (A[:, None] + Bc[:, None] * mx[None, :]).astype(np.float32)
        Qc = (Cc[:, None] * my[None, :]).astype(np.float32)
        Rd = (Dd[:, None] + Ed[:, None] * mx[None, :] + mx[None, :] ** 2
              ).astype(np.float32)
        Sd = (Fd[:, None] * my[None, :] + my[None, :] ** 2).astype(np.float32)
        prof.append((Pc, Qc, Rd, Sd))

    import ml_dtypes
    redp = np.zeros((NPTS, 63), dtype=ml_dtypes.bfloat16)
    redp[:, 31] = 1.0
    redm = np.zeros((NPTS, 63), dtype=np.float32)
    redm[:, 31] = 1.0

    in_maps = []
    for c in range(N_CORES):
        b = c // (N_CORES // B)
        r0 = (c % (N_CORES // B)) * ROWS_CORE
        Pc, Qc, Rd, Sd = prof[b]
        in_maps.append({
            "pc": np.ascontiguousarray(Pc[:, r0:r0 + ROWS_CORE]),
            "qc": Qc,
            "rd": np.ascontiguousarray(Rd[:, r0:r0 + ROWS_CORE]),
            "sd": Sd,
            "redp": redp,
            "redm": redm,
        })
    return in_maps


def kernel(contour: np.ndarray) -> np.ndarray:
    from concourse import bass_utils

    contour = np.asarray(contour, dtype=np.float32)
    if "nc" not in _compiled:
        _compiled["nc"] = _build()
    in_maps = _host_inputs(contour)
    res = bass_utils.run_bass_kernel_spmd(
        _compiled["nc"], in_maps, core_ids=list(range(N_CORES))).results

    mask = np.zeros((1, B, SIZE, SIZE), dtype=np.float32)
    for c in range(N_CORES):
        b = c // (N_CORES // B)
        r0 = (c % (N_CORES // B)) * ROWS_CORE
        mask[0, b, r0:r0 + ROWS_CORE, :] = (
            res[c]["out"].reshape(ROWS_CORE, SIZE))
    return mask


# revision 14
# speedup vs baseline: 218.2304x; 1.0039x over previous
"""Trainium2 Bass kernel for nn_Contour_to_mask (winding-number soft
rasterization of a 128-point contour into a (1, 2, 256, 256) f32 mask).

Math: for pixel m = (mx, my) = (i/256, j/256) and edge (c_n, c_{n+1}):
  cross_n(m) = (cy*cxn - cx*cyn) + (cyn-cy)*mx + (cx-cxn)*my
  dot_n(m)   = (cx*cxn + cy*cyn) - (cx+cxn)*mx - (cy+cyn)*my + mx^2 + my^2
Both are SEPARABLE into per-edge row/column profiles:
  cross[n, i, j] = Pc[n, i] + Qc[n, j];   dot[n, i, j] = Rd[n, i] + Sd[n, j].
  angle = arccos(clip(cos, -1+eps, 1-eps)) == pi/2 - arctan(clip(r, +-R1))
  with r = dot/|cross|, R1 = cot(arccos(1-eps)).
  contribution = tanh(1e5*cross)*angle; winding = |sum_n contrib|/2pi, clip.

Engine split per 2048-pixel (8-image-row) superblock (partitions = 128 edges):
  ACT:  4 cross row-builds (Identity w/ per-partition bias), s = tanh(1e5*
        cross) -> bf16, phi = arctan(rc). Tanh+Arctan share one table set.
  GPSIMD: 4 cross row-builds (tensor_scalar add), t2 = s*phi.
  DVE:  v = reciprocal_approx_fast(cross); per-row custom fused op
        rc = clip((Sd + Rd_i)*|v|, +-R1) that BUILDS dot inline (Sd tensor +
        per-partition scalar Rd_i) and guards NaN via select(m==m)
        (cross==+-0 -> v=NaN -> rc:=R1; s=tanh(0)=0 kills it, matching ref).
  PE:   reduction over the 128 edges via sliding-window one-hot lhsT matmuls
        into two PSUM tiles: accS = sum(s) (bf16 rhs, full-rate) and
        accT = sum(t2) (fp32 rhs); finale w = min(|pi/2*accS - accT|/2pi, 1).

Sharding: 8 cores; core c handles batch c//4, image rows [(c%4)*64, +64).
"""
import sys

sys.path.insert(0, "/opt/trn_rl_repo")

import numpy as np

SIZE = 256
K_TANH = 100000.0
EPS = 1e-5
B = 2
NPTS = 128
N_CORES = 8
PIX = SIZE * SIZE              # 65536
PIX_CORE = PIX * B // N_CORES  # 16384 pixels per core
ROWS_CORE = PIX_CORE // SIZE   # 64 image rows per core
BLK = 512                      # pixels per reduction block (one PSUM bank)
NBLK = PIX_CORE // BLK         # 32
SBLK = 2048                    # pixels per elementwise superblock (8 rows)
NSBLK = PIX_CORE // SBLK       # 8

_compiled = {}
_ops = {}


def _clip_bound():
    c = np.float64(np.float32(1.0 - EPS))
    return np.float32(c / np.sqrt(1.0 - c * c))


def _register_ops():
    """Register the two custom DVE ops (idempotent)."""
    if _ops:
        return _ops
    from concourse import dve_ops
    from concourse.dve_spec import (
        Spec, Src0, Src1, C0, C1, C2, Zero, maxx, minn, eq, select, lower)
    from concourse.dve_uop import DveOpSpec

    def reg(name, spec):
        if name in dve_ops._SUB_OPCODE_FOR_NAME:
            return next(op for op in dve_ops.OPS if op.name == name)
        row = dve_ops._CUSTOM_DVE_ROW_BASE + len(dve_ops.OPS)
        sha = {ver: DveOpSpec(name=name, opcode=row,
                              uops=lower(spec, ver=ver), rd1_en=True).sha(ver)
               for ver in ("v3", "v4")}
        op = dve_ops.DveOp(name, spec, subdim=False, uops_sha=sha)
        dve_ops.OPS.append(op)
        dve_ops.CUSTOM_DVE_SPECS[name] = spec
        dve_ops._SUB_OPCODE_FOR_NAME[name] = row
        return op

    # DOT_RMUL_CLIP: out = clip((Src1 + C0)*|Src0|, -C1, C1); NaN -> C1.
    # Src0 = v (recip of cross), Src1 = Sd column profile, C0 = Rd_i scalar.
    _d = Src1 + C0
    _av = maxx(Src0, Zero - Src0)
    _m = _d * _av
    _cl = minn(maxx(_m, Zero - C1), C1)
    _body = select(eq(_m, _m), _cl, C1)

    def _ref_rmul(in0, in1, s0, s1, imm2):
        m = (in1 + s0) * np.abs(in0)
        out = np.minimum(np.maximum(m, -s1), s1)
        return np.where(np.isnan(m), s1, out).astype(np.float32)

    _ops["rmul"] = reg("DOT_RMUL_CLIP", Spec(body=_body, reference=_ref_rmul))

    # FINALE: out = min(|Src0*C0 - Src1| * C1, C2)
    _fd = Src0 * C0 - Src1
    _fa = maxx(_fd, Zero - _fd)
    _fbody = minn(_fa * C1, C2)

    def _ref_fin(in0, in1, s0, s1, imm2):
        return np.minimum(np.abs(in0 * s0 - in1) * s1, imm2).astype(np.float32)

    _ops["fin"] = reg("WINDING_FINALE", Spec(body=_fbody, reference=_ref_fin))
    return _ops


def _build(repeat=1):
    import concourse.bacc as bacc
    import concourse.tile as tile
    import concourse.mybir as mybir

    AF = mybir.ActivationFunctionType
    ALU = mybir.AluOpType
    f32 = mybir.dt.float32
    bf16 = mybir.dt.bfloat16
    ops = _register_ops()

    nc = bacc.Bacc("TRN2", target_bir_lowering=False, debug=False,
                   num_devices=N_CORES)

    pc_d = nc.dram_tensor("pc", [NPTS, ROWS_CORE], f32, kind="ExternalInput").ap()
    qc_d = nc.dram_tensor("qc", [NPTS, SIZE], f32, kind="ExternalInput").ap()
    rd_d = nc.dram_tensor("rd", [NPTS, ROWS_CORE], f32, kind="ExternalInput").ap()
    sd_d = nc.dram_tensor("sd", [NPTS, SIZE], f32, kind="ExternalInput").ap()
    redp_d = nc.dram_tensor("redp", [NPTS, 63], bf16, kind="ExternalInput").ap()
    redm_d = nc.dram_tensor("redm", [NPTS, 63], f32, kind="ExternalInput").ap()
    out_d = nc.dram_tensor("out", [NBLK, BLK], f32, kind="ExternalOutput").ap()

    R1 = float(_clip_bound())
    RPB = SBLK // SIZE  # rows per superblock = 8
    BPB = SBLK // BLK   # reduction blocks per superblock = 4

    with tile.TileContext(nc) as tc:
        with tc.tile_pool(name="cst", bufs=1) as cst, \
             tc.tile_pool(name="work", bufs=3) as work, \
             tc.tile_pool(name="pacc", bufs=1, space="PSUM") as pacc:
            pc_t = cst.tile([NPTS, ROWS_CORE], f32, name="pc_t")
            qc_t = cst.tile([NPTS, SIZE], f32, name="qc_t")
            rd_t = cst.tile([NPTS, ROWS_CORE], f32, name="rd_t")
            sd_t = cst.tile([NPTS, SIZE], f32, name="sd_t")
            redp_t = cst.tile([NPTS, 63], bf16, name="redp_t")
            redm_t = cst.tile([NPTS, 63], f32, name="redm_t")
            nc.sync.dma_start(pc_t[:], pc_d[:])
            nc.sync.dma_start(qc_t[:], qc_d[:])
            nc.sync.dma_start(rd_t[:], rd_d[:])
            nc.sync.dma_start(sd_t[:], sd_d[:])
            nc.sync.dma_start(redp_t[:], redp_d[:])
            nc.sync.dma_start(redm_t[:], redm_d[:])

            accS = pacc.tile([NBLK, BLK], f32, name="accS")
            accT = pacc.tile([NBLK, BLK], f32, name="accT")

            for rep in range(repeat):
                for u in range(NSBLK):
                    cross = work.tile([NPTS, SBLK], f32, tag="cross",
                                      name=f"cross{rep}_{u}")
                    for h in range(RPB):
                        i = u * RPB + h  # local image row
                        hs = slice(h * SIZE, (h + 1) * SIZE)
                        if h % 2 == 1:
                            nc.gpsimd.tensor_scalar(
                                cross[:, hs], qc_t[:], pc_t[:, i:i + 1], None,
                                ALU.add)
                        else:
                            nc.vector.tensor_scalar(
                                cross[:, hs], qc_t[:], pc_t[:, i:i + 1], None,
                                ALU.add)

                    s = work.tile([NPTS, SBLK], bf16, tag="s", name=f"s{rep}_{u}")
                    for g in range(2):
                        gs = slice(g * (SBLK // 2), (g + 1) * (SBLK // 2))
                        nc.scalar.activation(s[:, gs], cross[:, gs], AF.Tanh,
                                             scale=K_TANH)

                    v = work.tile([NPTS, SBLK], f32, tag="v", name=f"v{rep}_{u}")
                    for g in range(2):
                        gs = slice(g * (SBLK // 2), (g + 1) * (SBLK // 2))
                        nc.vector.reciprocal_approx_fast(v[:, gs], cross[:, gs])

                    rc = work.tile([NPTS, SBLK], f32, tag="rc", name=f"rc{rep}_{u}")
                    for h in range(RPB):
                        i = u * RPB + h
                        hs = slice(h * SIZE, (h + 1) * SIZE)
                        nc.vector._custom_dve(
                            ops["rmul"], out=rc[:, hs], in0=v[:, hs],
                            in1=sd_t[:], s0=rd_t[:, i:i + 1], s1=R1)

                    phi = work.tile([NPTS, SBLK], f32, tag="phi",
                                    name=f"phi{rep}_{u}")
                    for g in range(2):
                        gs = slice(g * (SBLK // 2), (g + 1) * (SBLK // 2))
                        nc.scalar.activation(phi[:, gs], rc[:, gs], AF.Arctan)

                    t2 = work.tile([NPTS, SBLK], f32, tag="t2", name=f"t2{rep}_{u}")
                    for g in range(4):
                        gs = slice(g * (SBLK // 4), (g + 1) * (SBLK // 4))
                        nc.gpsimd.tensor_tensor(t2[:, gs], s[:, gs], phi[:, gs],
                                                ALU.mult)

                    for h in range(BPB):
                        j = BPB * u + h
                        hs = slice(h * BLK, (h + 1) * BLK)
                        lp = redp_t[:, 31 - j:63 - j]
                        lm = redm_t[:, 31 - j:63 - j]
                        nc.tensor.matmul(accS[:], lp, s[:, hs],
                                         start=(j == 0), stop=False)
                        nc.tensor.matmul(accT[:], lm, t2[:, hs],
                                         start=(j == 0),
                                         stop=(j == NBLK - 1 and
                                               rep == repeat - 1))

            tcopy = work.tile([NBLK, BLK], f32, tag="tcopy", name="tcopy")
            nc.vector.tensor_copy(tcopy[:], accT[:])
            w = work.tile([NBLK, BLK], f32, tag="w", name="w")
            nc.vector._custom_dve(
                ops["fin"], out=w[:], in0=accS[:], in1=tcopy[:],
                s0=float(np.float32(np.pi / 2)),
                s1=float(np.float32(1.0 / (2.0 * np.pi))), imm2=1.0)
            nc.sync.dma_start(out_d[:], w[:])

    nc.compile()
    return nc


def _host_inputs(contour: np.ndarray):
    """Per-core in_maps from the full (B, NPTS, 2) contour."""
    mx = (np.arange(SIZE) / SIZE).astype(np.float64)   # i profile
    my = (np.arange(SIZE) / SIZE).astype(np.float64)   # j profile

    prof = []
    for b in range(B):
        cx = contour[b, :, 0].astype(np.float64)
        cy = contour[b, :, 1].astype(np.float64)
        cxn = np.roll(cx, -1)
        cyn = np.roll(cy, -1)
        A = cy * cxn - cx * cyn
        Bc = cyn - cy
        Cc = cx - cxn
        Dd = cx * cxn + cy * cyn
        Ed = -(cx + cxn)
        Fd = -(cy + cyn)
        Pc = (A[:, None] + Bc[:, None] * mx[None, :]).astype(np.float32)
        Qc = (Cc[:, None] * my[None, :]).astype(np.float32)
        Rd = (Dd[:, None] + Ed[:, None] * mx[None, :] + mx[None, :] ** 2
              ).astype(np.float32)
        Sd = (Fd[:, None] * my[None, :] + my[None, :] ** 2).astype(np.float32)
        prof.append((Pc, Qc, Rd, Sd))

    import ml_dtypes
    redp = np.zeros((NPTS, 63), dtype=ml_dtypes.bfloat16)
    redp[:, 31] = 1.0
    redm = np.zeros((NPTS, 63), dtype=np.float32)
    redm[:, 31] = 1.0

    in_maps = []
    for c in range(N_CORES):
        b = c // (N_CORES // B)
        r0 = (c % (N_CORES // B)) * ROWS_CORE
        Pc, Qc, Rd, Sd = prof[b]
        in_maps.append({
            "pc": np.ascontiguousarray(Pc[:, r0:r0 + ROWS_CORE]),
            "qc": Qc,
            "rd": np.ascontiguousarray(Rd[:, r0:r0 + ROWS_CORE]),
            "sd": Sd,
            "redp": redp,
            "redm": redm,
        })
    return in_maps


def kernel(contour: np.ndarray) -> np.ndarray:
    from concourse import bass_utils

    contour = np.asarray(contour, dtype=np.float32)
    if "nc" not in _compiled:
        _compiled["nc"] = _build()
    in_maps = _host_inputs(contour)
    res = bass_utils.run_bass_kernel_spmd(
        _compiled["nc"], in_maps, core_ids=list(range(N_CORES))).results

    mask = np.zeros((1, B, SIZE, SIZE), dtype=np.float32)
    for c in range(N_CORES):
        b = c // (N_CORES // B)
        r0 = (c % (N_CORES // B)) * ROWS_CORE
        mask[0, b, r0:r0 + ROWS_CORE, :] = (
            res[c]["out"].reshape(ROWS_CORE, SIZE))
    return mask


# revision 15
# speedup vs baseline: 219.2824x; 1.0048x over previous
"""Trainium2 Bass kernel for nn_Contour_to_mask (winding-number soft
rasterization of a 128-point contour into a (1, 2, 256, 256) f32 mask).

Math: for pixel m = (mx, my) = (i/256, j/256) and edge (c_n, c_{n+1}):
  cross_n(m) = (cy*cxn - cx*cyn) + (cyn-cy)*mx + (cx-cxn)*my
  dot_n(m)   = (cx*cxn + cy*cyn) - (cx+cxn)*mx - (cy+cyn)*my + mx^2 + my^2
Both are SEPARABLE into per-edge row/column profiles:
  cross[n, i, j] = Pc[n, i] + Qc[n, j];   dot[n, i, j] = Rd[n, i] + Sd[n, j].
  angle = arccos(clip(cos, -1+eps, 1-eps)) == pi/2 - arctan(clip(r, +-R1))
  with r = dot/|cross|, R1 = cot(arccos(1-eps)).
  contribution = tanh(1e5*cross)*angle; winding = |sum_n contrib|/2pi, clip.

Engine split per 2048-pixel (8-image-row) superblock (partitions = 128 edges):
  ACT:  4 cross row-builds (Identity w/ per-partition bias), s = tanh(1e5*
        cross) -> bf16, phi = arctan(rc). Tanh+Arctan share one table set.
  GPSIMD: 4 cross row-builds (tensor_scalar add), t2 = s*phi.
  DVE:  v = reciprocal_approx_fast(cross); per-row custom fused op
        rc = clip((Sd + Rd_i)*|v|, +-R1) that BUILDS dot inline (Sd tensor +
        per-partition scalar Rd_i) and guards NaN via select(m==m)
        (cross==+-0 -> v=NaN -> rc:=R1; s=tanh(0)=0 kills it, matching ref).
  PE:   reduction over the 128 edges via sliding-window one-hot lhsT matmuls
        into two PSUM tiles: accS = sum(s) (bf16 rhs, full-rate) and
        accT = sum(t2) (fp32 rhs); finale w = min(|pi/2*accS - accT|/2pi, 1).

Sharding: 8 cores; core c handles batch c//4, image rows [(c%4)*64, +64).
"""
import sys

sys.path.insert(0, "/opt/trn_rl_repo")

import numpy as np

SIZE = 256
K_TANH = 100000.0
EPS = 1e-5
B = 2
NPTS = 128
N_CORES = 8
PIX = SIZE * SIZE              # 65536
PIX_CORE = PIX * B // N_CORES  # 16384 pixels per core
ROWS_CORE = PIX_CORE // SIZE   # 64 image rows per core
BLK = 512                      # pixels per reduction block (one PSUM bank)
NBLK = PIX_CORE // BLK         # 32
SBLK = 2048                    # pixels per elementwise superblock (8 rows)
NSBLK = PIX_CORE // SBLK       # 8

_compiled = {}
_ops = {}


def _clip_bound():
    c = np.float64(np.float32(1.0 - EPS))
    return np.float32(c / np.sqrt(1.0 - c * c))


def _register_ops():
    """Register the two custom DVE ops (idempotent)."""
    if _ops:
        return _ops
    from concourse import dve_ops
    from concourse.dve_spec import (
        Spec, Src0, Src1, C0, C1, C2, Zero, maxx, minn, eq, select, lower)
    from concourse.dve_uop import DveOpSpec

    def reg(name, spec):
        if name in dve_ops._SUB_OPCODE_FOR_NAME:
            return next(op for op in dve_ops.OPS if op.name == name)
        row = dve_ops._CUSTOM_DVE_ROW_BASE + len(dve_ops.OPS)
        sha = {ver: DveOpSpec(name=name, opcode=row,
                              uops=lower(spec, ver=ver), rd1_en=True).sha(ver)
               for ver in ("v3", "v4")}
        op = dve_ops.DveOp(name, spec, subdim=False, uops_sha=sha)
        dve_ops.OPS.append(op)
        dve_ops.CUSTOM_DVE_SPECS[name] = spec
        dve_ops._SUB_OPCODE_FOR_NAME[name] = row
        return op

    # DOT_RMUL_CLIP: out = clip((Src1 + C0)*|Src0|, -C1, C1); NaN -> C1.
    # Src0 = v (recip of cross), Src1 = Sd column profile, C0 = Rd_i scalar.
    _d = Src1 + C0
    _av = maxx(Src0, Zero - Src0)
    _m = _d * _av
    _cl = minn(maxx(_m, Zero - C1), C1)
    _body = select(eq(_m, _m), _cl, C1)

    def _ref_rmul(in0, in1, s0, s1, imm2):
        m = (in1 + s0) * np.abs(in0)
        out = np.minimum(np.maximum(m, -s1), s1)
        return np.where(np.isnan(m), s1, out).astype(np.float32)

    _ops["rmul"] = reg("DOT_RMUL_CLIP", Spec(body=_body, reference=_ref_rmul))

    # FINALE: out = min(|Src0*C0 - Src1| * C1, C2)
    _fd = Src0 * C0 - Src1
    _fa = maxx(_fd, Zero - _fd)
    _fbody = minn(_fa * C1, C2)

    def _ref_fin(in0, in1, s0, s1, imm2):
        return np.minimum(np.abs(in0 * s0 - in1) * s1, imm2).astype(np.float32)

    _ops["fin"] = reg("WINDING_FINALE", Spec(body=_fbody, reference=_ref_fin))
    return _ops


def _build(repeat=1):
    import concourse.bacc as bacc
    import concourse.tile as tile
    import concourse.mybir as mybir

    AF = mybir.ActivationFunctionType
    ALU = mybir.AluOpType
    f32 = mybir.dt.float32
    bf16 = mybir.dt.bfloat16
    ops = _register_ops()

    nc = bacc.Bacc("TRN2", target_bir_lowering=False, debug=False,
                   num_devices=N_CORES)

    pc_d = nc.dram_tensor("pc", [NPTS, ROWS_CORE], f32, kind="ExternalInput").ap()
    qc_d = nc.dram_tensor("qc", [NPTS, SIZE], f32, kind="ExternalInput").ap()
    rd_d = nc.dram_tensor("rd", [NPTS, ROWS_CORE], f32, kind="ExternalInput").ap()
    sd_d = nc.dram_tensor("sd", [NPTS, SIZE], f32, kind="ExternalInput").ap()
    redp_d = nc.dram_tensor("redp", [NPTS, 63], bf16, kind="ExternalInput").ap()
    redm_d = nc.dram_tensor("redm", [NPTS, 63], f32, kind="ExternalInput").ap()
    out_d = nc.dram_tensor("out", [NBLK, BLK], f32, kind="ExternalOutput").ap()

    R1 = float(_clip_bound())
    RPB = SBLK // SIZE  # rows per superblock = 8
    BPB = SBLK // BLK   # reduction blocks per superblock = 4

    with tile.TileContext(nc) as tc:
        with tc.tile_pool(name="cst", bufs=1) as cst, \
             tc.tile_pool(name="work", bufs=3) as work, \
             tc.tile_pool(name="pacc", bufs=1, space="PSUM") as pacc:
            pc_t = cst.tile([NPTS, ROWS_CORE], f32, name="pc_t")
            qc_t = cst.tile([NPTS, SIZE], f32, name="qc_t")
            rd_t = cst.tile([NPTS, ROWS_CORE], f32, name="rd_t")
            sd_t = cst.tile([NPTS, SIZE], f32, name="sd_t")
            redp_t = cst.tile([NPTS, 63], bf16, name="redp_t")
            redm_t = cst.tile([NPTS, 63], f32, name="redm_t")
            nc.sync.dma_start(pc_t[:], pc_d[:])
            nc.sync.dma_start(qc_t[:], qc_d[:])
            nc.sync.dma_start(rd_t[:], rd_d[:])
            nc.sync.dma_start(sd_t[:], sd_d[:])
            nc.sync.dma_start(redp_t[:], redp_d[:])
            nc.sync.dma_start(redm_t[:], redm_d[:])

            accS = pacc.tile([NBLK, BLK], f32, name="accS")
            accT = pacc.tile([NBLK, BLK], f32, name="accT")

            for rep in range(repeat):
                for u in range(NSBLK):
                    cross = work.tile([NPTS, SBLK], f32, tag="cross",
                                      name=f"cross{rep}_{u}")
                    for h in range(RPB):
                        i = u * RPB + h  # local image row
                        hs = slice(h * SIZE, (h + 1) * SIZE)
                        if h % 2 == 1:
                            nc.gpsimd.tensor_scalar(
                                cross[:, hs], qc_t[:], pc_t[:, i:i + 1], None,
                                ALU.add)
                        else:
                            nc.vector.tensor_scalar(
                                cross[:, hs], qc_t[:], pc_t[:, i:i + 1], None,
                                ALU.add)

                    s = work.tile([NPTS, SBLK], bf16, tag="s", name=f"s{rep}_{u}")
                    for g in range(2):
                        gs = slice(g * (SBLK // 2), (g + 1) * (SBLK // 2))
                        nc.scalar.activation(s[:, gs], cross[:, gs], AF.Tanh,
                                             scale=K_TANH)

                    v = work.tile([NPTS, SBLK], f32, tag="v", name=f"v{rep}_{u}")
                    nc.vector.reciprocal_approx_fast(v[:], cross[:])

                    rc = work.tile([NPTS, SBLK], f32, tag="rc", name=f"rc{rep}_{u}")
                    for h in range(RPB):
                        i = u * RPB + h
                        hs = slice(h * SIZE, (h + 1) * SIZE)
                        nc.vector._custom_dve(
                            ops["rmul"], out=rc[:, hs], in0=v[:, hs],
                            in1=sd_t[:], s0=rd_t[:, i:i + 1], s1=R1)

                    phi = work.tile([NPTS, SBLK], f32, tag="phi",
                                    name=f"phi{rep}_{u}")
                    for g in range(2):
                        gs = slice(g * (SBLK // 2), (g + 1) * (SBLK // 2))
                        nc.scalar.activation(phi[:, gs], rc[:, gs], AF.Arctan)

                    t2 = work.tile([NPTS, SBLK], f32, tag="t2", name=f"t2{rep}_{u}")
                    for g in range(4):
                        gs = slice(g * (SBLK // 4), (g + 1) * (SBLK // 4))
                        nc.gpsimd.tensor_tensor(t2[:, gs], s[:, gs], phi[:, gs],
                                                ALU.mult)

                    for h in range(BPB):
                        j = BPB * u + h
                        hs = slice(h * BLK, (h + 1) * BLK)
                        lp = redp_t[:, 31 - j:63 - j]
                        lm = redm_t[:, 31 - j:63 - j]
                        nc.tensor.matmul(accS[:], lp, s[:, hs],
                                         start=(j == 0), stop=False)
                        nc.tensor.matmul(accT[:], lm, t2[:, hs],
                                         start=(j == 0),
                                         stop=(j == NBLK - 1 and
                                               rep == repeat - 1))

            tcopy = work.tile([NBLK, BLK], f32, tag="tcopy", name="tcopy")
            nc.vector.tensor_copy(tcopy[:], accT[:])
            w = work.tile([NBLK, BLK], f32, tag="w", name="w")
            nc.vector._custom_dve(
                ops["fin"], out=w[:], in0=accS[:], in1=tcopy[:],
                s0=float(np.float32(np.pi / 2)),
                s1=float(np.float32(1.0 / (2.0 * np.pi))), imm2=1.0)
            nc.sync.dma_start(out_d[:], w[:])

    nc.compile()
    return nc


def _host_inputs(contour: np.ndarray):
    """Per-core in_maps from the full (B, NPTS, 2) contour."""
    mx = (np.arange(SIZE) / SIZE).astype(np.float64)   # i profile
    my = (np.arange(SIZE) / SIZE).astype(np.float64)   # j profile

    prof = []
    for b in range(B):
        cx = contour[b, :, 0].astype(np.float64)
        cy = contour[b, :, 1].astype(np.float64)
        cxn = np.roll(cx, -1)
        cyn = np.roll(cy, -1)
        A = cy * cxn - cx * cyn
        Bc = cyn - cy
        Cc = cx - cxn
        Dd = cx * cxn + cy * cyn
        Ed = -(cx + cxn)
        Fd = -(cy + cyn)
        Pc = (A[:, None] + Bc[:, None] * mx[None, :]).astype(np.float32)
        Qc = (Cc[:, None] * my[None, :]).astype(np.float32)
        Rd = (Dd[:, None] + Ed[:, None] * mx[None, :] + mx[None, :] ** 2
              ).astype(np.float32)
        Sd = (Fd[:, None] * my[None, :] + my[None, :] ** 2).astype(np.float32)
        prof.append((Pc, Qc, Rd, Sd))

    import ml_dtypes
    redp = np.zeros((NPTS, 63), dtype=ml_dtypes.bfloat16)
    redp[:, 31] = 1.0
    redm = np.zeros((NPTS, 63), dtype=np.float32)
    redm[:, 31] = 1.0

    in_maps = []
    for c in range(N_CORES):
        b = c // (N_CORES // B)
        r0 = (c % (N_CORES // B)) * ROWS_CORE
        Pc, Qc, Rd, Sd = prof[b]
        in_maps.append({
            "pc": np.ascontiguousarray(Pc[:, r0:r0 + ROWS_CORE]),
            "qc": Qc,
            "rd": np.ascontiguousarray(Rd[:, r0:r0 + ROWS_CORE]),
            "sd": Sd,
            "redp": redp,
            "redm": redm,
        })
    return in_maps


def kernel(contour: np.ndarray) -> np.ndarray:
    from concourse import bass_utils

    contour = np.asarray(contour, dtype=np.float32)
    if "nc" not in _compiled:
        _compiled["nc"] = _build()
    in_maps = _host_inputs(contour)
    res = bass_utils.run_bass_kernel_spmd(
        _compiled["nc"], in_maps, core_ids=list(range(N_CORES))).results

    mask = np.zeros((1, B, SIZE, SIZE), dtype=np.float32)
    for c in range(N_CORES):
        b = c // (N_CORES // B)
        r0 = (c % (N_CORES // B)) * ROWS_CORE
        mask[0, b, r0:r0 + ROWS_CORE, :] = (
            res[c]["out"].reshape(ROWS_CORE, SIZE))
    return mask
